# revision 4
# baseline (speedup 1.0000x reference)
"""Trainium2 Bass kernel for nn_AugmentPipe (gated flips / 90-degree rots /
reflect-pad integer translation), data-parallel over the batch on 8 cores.

The whole pipeline is a per-sample separable gather:
    out[y, x, c] = in[a[y], b[x], c]            (no transpose), or
    out[y, x, c] = in[a[x], b[y], c]            (rot 90/270)
where a, b are per-sample index vectors and the transpose flag comes from
rot_w. All per-sample control (flips, rotation, translation) is folded into
(a, b, transpose) on the host; the device program is identical for every
sample so one SPMD NEFF serves all 8 cores:

  1. dma_gather: rows in[a[k], :] -> SBUF, two images per gather (halves
     gpsimd descriptor-generation instructions)
  2. column gather by b on DVE as 2 fixed copies (main + edge) whose
     source/dest element offsets are per-image registers unpacked from 2
     packed parameter words (vs 5 words: reg_load costs ~360ns/reg); b is
     always one +-1 main run (>=224) plus at most one +-1 edge run (<=32)
     from reflection padding, so padded fixed-length copies + overwrite
     order realize any b
  3. PE fp32 transpose (exact pass-through) of the gathered tile, always
  4. two cond-predicated DMA stores on SEPARATE HWDGE rings: the
     untransposed store on sync (waits only on DVE), the transposed store
     on scalar (which also does the PSUM->SBUF copies, so it follows them
     with no cross-engine wait). Keeps either store from FIFO-blocking the
     other ring behind the full per-image compute chain.
"""
import sys

for _p in ("/opt/trn_rl_repo",):
    if _p not in sys.path:
        sys.path.insert(0, _p)

import numpy as np

N_CORES = 8
N, H, W, C = 128, 256, 256, 3
PER_CORE = N // N_CORES
PAIRS = PER_CORE // 2
ROW_ELEMS = W * C  # 768
PAD = 96  # 32 pixels of slack around each data block (elements)

# M1 (gather target) free-dim layout, in elements: a PAIR of images
#   [96 lead pad][768 A.h0][768 A.h1][768 B.h0][768 B.h1][96 tail pad]
M1_LEAD = PAD
M1_HSTRIDE = ROW_ELEMS
M1_IMG = 2 * ROW_ELEMS  # per-image block inside the pair tile
M1_W = PAD + 4 * ROW_ELEMS + PAD  # 3264

# N (column-gathered) free-dim layout: [96 lead][768 h0][96 shared pad]
# [768 h1][96 tail]. Edge copies for images with no edge run dump into the
# lead pad (offset 0 writes [0,96) and [864,960) - both pads).
N_LEAD = PAD
N_HSTRIDE = ROW_ELEMS + PAD  # 864
N_W = PAD + 2 * ROW_ELEMS + 3 * PAD  # 1824

EDGE_PIX = 32
# params layout (int32 words):
#   [2*PER_CORE packed dve words][PER_CORE/4 cn-packed][PER_CORE/4 ct-packed]
NPAR_WORDS = 2 * PER_CORE + 2 * (PER_CORE // 4)


def _derive_maps(xflip_w, xflip_gate, yflip_w, yflip_gate, rot_w, rot_gate,
                 trans_w, trans_gate):
    """Replicate the reference gate logic; return (a[N,256], b[N,256], tr[N])."""
    f32 = np.float32
    n = xflip_w.shape[0]
    wx = np.where(np.asarray(xflip_gate).reshape(n) < f32(1.0),
                  np.asarray(xflip_w).reshape(n), 0)
    wy = np.where(np.asarray(yflip_gate).reshape(n) < f32(1.0),
                  np.asarray(yflip_w).reshape(n), 0)
    rw = np.where(np.asarray(rot_gate).reshape(n) < f32(1.0),
                  np.asarray(rot_w).reshape(n), 0)
    tw = np.asarray(trans_w, dtype=np.float32).reshape(2, n) * f32(2.0) - f32(1.0)
    tg = np.asarray(trans_gate).reshape(n)
    tw = np.where(tg[None, :] < f32(1.0), tw, f32(0.0)).astype(np.float32)
    tx = np.round((tw[0] * f32(W)) * f32(0.125)).astype(np.int32)
    ty = np.round((tw[1] * f32(H)) * f32(0.125)).astype(np.int32)

    idx = np.arange(W)
    xi = (W - 1) - np.abs((W - 1) - (idx[None, :] - tx[:, None]) % (2 * W - 2))
    yi = (H - 1) - np.abs((H - 1) - (idx[None, :] + ty[:, None]) % (2 * H - 2))

    xftot = (wx == 1) ^ ((rw == 1) | (rw == 2))
    yftot = (wy == 1) ^ ((rw == 2) | (rw == 3))
    tr = (rw == 1) | (rw == 3)

    a = np.where(tr[:, None], xi, yi)
    a = np.where(yftot[:, None], (H - 1) - a, a)
    b = np.where(tr[:, None], yi, xi)
    b = np.where(xftot[:, None], (W - 1) - b, b)
    return a.astype(np.int64), b.astype(np.int64), tr


def _fit_template(b, base):
    """Fit b (one +-1 main run >=224 plus <=1 edge run <=32) to the fixed
    2-copy template; `base` is the image's element offset inside its pair
    tile. Returns 2 packed int32 words:
      word0 = m_src | m_dst << 16
      word1 = e_src | e_dst << 16 | R << 31     (R=1 -> descending main)
    """
    d = np.diff(b)
    assert np.all(np.abs(d) == 1), b
    change = np.nonzero(d[1:] != d[:-1])[0]
    assert len(change) <= 1, b
    if len(change) == 0:
        runs = [(0, W, int(d[0]))]
    else:
        # the pivot position can belong to either run; pick the split whose
        # short run is <= EDGE_PIX
        c0 = int(change[0])
        runs = None
        for cut in (c0 + 1, c0 + 2):
            r = [(0, cut, int(d[0])), (cut, W, int(d[cut]))]
            lens = sorted(e - s for s, e, _ in r)
            if lens[0] <= EDGE_PIX and lens[1] >= W - EDGE_PIX:
                runs = r
                break
        assert runs is not None, (b, c0)
    if len(runs) == 1:
        main, edge = runs[0], None
    else:
        r0, r1 = runs
        if (r0[1] - r0[0]) >= (r1[1] - r1[0]):
            main, edge = r0, r1
        else:
            main, edge = r1, r0
    mp, mq, md = main
    assert mq - mp >= W - EDGE_PIX, (b, runs)

    # main direction decides the branch: R=0 -> asc main + desc edge,
    # R=1 -> desc main + asc edge
    R = 0 if md == 1 else 1
    m_src = base + 3 * int(b[mp])
    m_dst = N_LEAD + 3 * mp

    if edge is not None:
        ep, eq, ed = edge
        assert eq - ep <= EDGE_PIX and ed == -md, (b, runs)
        if ep == 0:
            wstart = eq - EDGE_PIX  # head edge: window [eq-32, eq)
        else:
            assert eq == W, (b, runs)
            wstart = ep             # tail edge: window [ep, ep+32)
        v0 = int(b[ep]) + ed * (wstart - ep)  # value at window start
        e_src = base + 3 * v0
        e_dst = N_LEAD + 3 * wstart
        assert e_src >= 0 and e_dst > 0, (b, runs, e_src, e_dst)
    else:
        # taken branch's edge copy still runs; dump into the N pads at 0
        e_src = base if md == -1 else base + 3 * (EDGE_PIX - 1)
        e_dst = 0

    for v in (m_src, m_dst, e_src, e_dst):
        assert 0 <= v < (1 << 15), (m_src, m_dst, e_src, e_dst)
    w0 = m_src | (m_dst << 16)
    w1 = e_src | (e_dst << 16) | (R << 31)
    return [w0, w1]


def _pack_gather_idx(a_core):
    """a_core: [PER_CORE, 256] row indices -> int16 [128, 32*PAIRS] in
    dma_gather layout: per pair, 512 indices (image A rows then 256+B rows),
    index i at partition i%16, col i//16, replicated to all 8 gpsimd core
    partition groups."""
    out = np.zeros((128, 32 * PAIRS), np.int16)
    for pr in range(PAIRS):
        v = np.concatenate([a_core[2 * pr], 256 + a_core[2 * pr + 1]])
        blk = v.astype(np.int16).reshape(32, 16).T  # [p=i%16, s=i//16]
        for g in range(8):
            out[16 * g:16 * (g + 1), 32 * pr:32 * (pr + 1)] = blk
    return out


_NC_CACHE = {}


def _build_module(coresim_pads=False):
    key = ("nc", coresim_pads)
    if key in _NC_CACHE:
        return _NC_CACHE[key]
    import concourse.bacc as bacc
    import concourse.bass as bass
    import concourse.mybir as mybir
    import concourse.tile as tile
    from concourse.ap import AP

    DT = mybir.dt.float32
    I32 = mybir.dt.int32
    ALU = mybir.AluOpType
    nc = bacc.Bacc(None, num_swdge_queues=2)
    images = nc.dram_tensor("images", [PER_CORE, H, W, C], DT, kind="ExternalInput")
    identity_in = nc.dram_tensor("identity_in", [128, 128], DT, kind="ExternalInput")
    gidx = nc.dram_tensor("gidx", [128, 32 * PAIRS], mybir.dt.int16,
                          kind="ExternalInput")
    params = nc.dram_tensor("params", [1, NPAR_WORDS], I32, kind="ExternalInput")
    out = nc.dram_tensor("out", [PER_CORE, H, W, C], DT, kind="ExternalOutput")

    img_elems = H * W * C

    with tile.TileContext(nc) as tc:
        with (
            tc.tile_pool(name="const", bufs=1) as const_pool,
            tc.tile_pool(name="m1", bufs=5) as m1_pool,
            tc.tile_pool(name="ncg", bufs=8) as n_pool,
            tc.tile_pool(name="tt", bufs=6) as t_pool,
            tc.tile_pool(name="psum", bufs=4, space="PSUM") as psum_pool,
        ):
            # warmup: a throwaway 16-row gather issued first so the gpsimd
            # ucode library load (ModifyPoolConfig + ~8us DMA) overlaps the
            # preamble instead of gating the first real gather
            warm_idx = const_pool.tile([128, 1], mybir.dt.int16)
            nc.gpsimd.memset(warm_idx[:], 0)
            warm_out = const_pool.tile([128, 1, ROW_ELEMS], DT)
            nc.gpsimd.dma_gather(
                warm_out[:], AP(images[:].tensor, 0, [[ROW_ELEMS, H], [1, ROW_ELEMS]]),
                warm_idx[:], num_idxs=16, num_idxs_reg=16, elem_size=ROW_ELEMS,
                queue_num=0, single_packet=False)

            idx_t = const_pool.tile([128, 32 * PAIRS], mybir.dt.int16)
            nc.sync.dma_start(idx_t[:], gidx[:])
            par_t = const_pool.tile([1, NPAR_WORDS], I32)
            nc.sync.dma_start(par_t[:], params[:])
            ident = const_pool.tile([128, 128], DT)
            nc.sync.dma_start(ident[:], identity_in[:])

            dve = nc.vector.engine
            act = nc.scalar.engine
            sp = nc.sync.engine

            # per-4-image packed store-cond registers, one stream per engine
            cn_group = [None]
            ct_group = [None]

            m1_tiles = [None] * PAIRS

            def emit_gather(pr):
                m1 = m1_pool.tile([128, M1_W], DT, tag="m1")
                m1_tiles[pr] = m1
                if coresim_pads:
                    nc.gpsimd.memset(m1[:, 0:M1_LEAD], 0.0)
                    nc.gpsimd.memset(m1[:, M1_W - PAD:M1_W], 0.0)
                src = AP(images[:].tensor, 2 * pr * img_elems,
                         [[ROW_ELEMS, 2 * H], [1, ROW_ELEMS]])
                gout = m1[:, M1_LEAD:M1_LEAD + 4 * ROW_ELEMS].rearrange(
                    "p (h e) -> p h e", h=4)
                nc.gpsimd.dma_gather(
                    gout, src, idx_t[:, 32 * pr:32 * (pr + 1)],
                    num_idxs=2 * H, num_idxs_reg=2 * H, elem_size=ROW_ELEMS,
                    queue_num=pr % 2, single_packet=False)

            emit_gather(0)
            emit_gather(1)

            for i in range(PER_CORE):
                pr = i // 2
                if i % 2 == 0 and pr + 2 < PAIRS:
                    emit_gather(pr + 2)
                m1 = m1_tiles[pr]
                base = M1_LEAD + (i % 2) * M1_IMG

                # --- 2. column gather by b: M1 -> Ntile (2 reg-offset
                # copies selected by the R sign-bit branch) ---
                ntile = n_pool.tile([128, N_W], DT, tag="ncg")
                m1t, ntt = m1[:].tensor, ntile[:].tensor
                p_m1 = [M1_W, 128]
                p_n = [N_W, 128]
                w0 = nc.alloc_register(dve, f"cg{i}_w0")
                w1 = nc.alloc_register(dve, f"cg{i}_w1")
                nc.vector.reg_load([w0, w1], par_t[0:1, 2 * i:2 * i + 2])
                m_src = nc.alloc_register(dve, f"cg{i}_ms")
                m_dst = nc.alloc_register(dve, f"cg{i}_md")
                e_src = nc.alloc_register(dve, f"cg{i}_es")
                e_dst = nc.alloc_register(dve, f"cg{i}_ed")
                nc.vector.reg_alu(m_src, bass.RuntimeValue(w0), 0xFFFF,
                                  ALU.bitwise_and)
                nc.vector.reg_alu(m_dst, bass.RuntimeValue(w0), 16,
                                  ALU.logical_shift_right)
                nc.vector.reg_alu(e_src, bass.RuntimeValue(w1), 0xFFFF,
                                  ALU.bitwise_and)
                nc.vector.reg_alu(e_dst, bass.RuntimeValue(w1), 16,
                                  ALU.logical_shift_right)
                nc.vector.reg_alu(e_dst, bass.RuntimeValue(e_dst), 0x7FFF,
                                  ALU.bitwise_and)
                with tc.If(bass.RuntimeValue(w1) >= 0) as cmp:
                    # R=0: ascending main + descending edge
                    nc.vector.tensor_copy(
                        AP(ntt, m_dst, [p_n, [N_HSTRIDE, 2], [1, ROW_ELEMS]]),
                        AP(m1t, m_src, [p_m1, [M1_HSTRIDE, 2], [1, ROW_ELEMS]]))
                    nc.vector.tensor_copy(
                        AP(ntt, e_dst, [p_n, [N_HSTRIDE, 2], [1, 3 * EDGE_PIX]]),
                        AP(m1t, e_src, [p_m1, [M1_HSTRIDE, 2], [-3, EDGE_PIX], [1, C]]))
                with cmp.Else():
                    # R=1: descending main + ascending edge
                    nc.vector.tensor_copy(
                        AP(ntt, m_dst, [p_n, [N_HSTRIDE, 2], [1, ROW_ELEMS]]),
                        AP(m1t, m_src, [p_m1, [M1_HSTRIDE, 2], [-3, W], [1, C]]))
                    nc.vector.tensor_copy(
                        AP(ntt, e_dst, [p_n, [N_HSTRIDE, 2], [1, 3 * EDGE_PIX]]),
                        AP(m1t, e_src, [p_m1, [M1_HSTRIDE, 2], [1, 3 * EDGE_PIX]]))

                # --- 3. pixel transpose Ntile -> Ttile via PE (exact fp32) ---
                # 6 channel transposes interleave into one 2-bank PSUM tile
                # (hu blocks at 512-elem, i.e. bank-aligned, offsets); a single
                # strided copy on the scalar engine moves each [128, 2, 384]
                # double-block out
                ttile = t_pool.tile([128, 2, ROW_ELEMS], DT, tag="tt")
                ttt = ttile[:].tensor
                for hk in range(2):
                    pt = psum_pool.tile([128, 1024], DT, tag="pt")
                    ptt = pt[:].tensor
                    for hu in range(2):
                        for c in range(C):
                            stat = AP(ntt, N_LEAD + hk * N_HSTRIDE + 3 * (hu * 128) + c,
                                      [p_n, [3, 128]])
                            nc.tensor.transpose(
                                AP(ptt, 512 * hu + c, [[1024, 128], [3, 128]]),
                                stat, ident[:])
                    t0 = 3 * (hk * 128)
                    nc.scalar.copy(
                        AP(ttt, t0, [[2 * ROW_ELEMS, 128], [ROW_ELEMS, 2],
                                     [1, 3 * 128]]),
                        AP(ptt, 0, [[1024, 128], [512, 2], [1, 3 * 128]]))

                # --- 4. predicated stores, one per HWDGE ring ---
                dram_out = AP(out[:].tensor, i * img_elems,
                              [[ROW_ELEMS, 128], [128 * ROW_ELEMS, 2], [1, ROW_ELEMS]])
                n_src = AP(ntt, N_LEAD, [p_n, [N_HSTRIDE, 2], [1, ROW_ELEMS]])
                g, sl = i // 4, i % 4
                if sl == 0:
                    cn_group[0] = nc.alloc_register(sp, f"cn_g{g}")
                    nc.sync.reg_load([cn_group[0]],
                                     par_t[0:1, 2 * PER_CORE + g:2 * PER_CORE + g + 1])
                    ct_group[0] = nc.alloc_register(act, f"ct_g{g}")
                    nc.scalar.reg_load(
                        [ct_group[0]],
                        par_t[0:1, 2 * PER_CORE + PER_CORE // 4 + g:
                              2 * PER_CORE + PER_CORE // 4 + g + 1])
                cn_reg = nc.alloc_register(sp, f"cn_{i}")
                nc.sync.reg_alu(cn_reg, bass.RuntimeValue(cn_group[0]), 8 * sl,
                                ALU.logical_shift_right)
                nc.sync.reg_alu(cn_reg, bass.RuntimeValue(cn_reg), 1,
                                ALU.bitwise_and)
                cn = nc.sync.snap(cn_reg, min_val=0, max_val=1)
                nc.sync.dma_start(dram_out, n_src, cond=cn)

                ct_reg = nc.alloc_register(act, f"ct_{i}")
                nc.scalar.reg_alu(ct_reg, bass.RuntimeValue(ct_group[0]), 8 * sl,
                                  ALU.logical_shift_right)
                nc.scalar.reg_alu(ct_reg, bass.RuntimeValue(ct_reg), 1,
                                  ALU.bitwise_and)
                ct = nc.scalar.snap(ct_reg, min_val=0, max_val=1)
                nc.scalar.dma_start(dram_out.copy(), ttile[:], cond=ct)

    nc.finalize()
    _NC_CACHE[key] = nc
    return nc


def _make_in_maps(images, a, b, tr):
    ident = np.eye(128, dtype=np.float32)
    in_maps = []
    for core in range(N_CORES):
        s = core * PER_CORE
        par = np.zeros((1, NPAR_WORDS), np.int32)
        for i in range(PER_CORE):
            base = M1_LEAD + (i % 2) * M1_IMG
            par[0, 2 * i:2 * i + 2] = np.array(
                _fit_template(b[s + i], base), np.uint32).astype(np.int32)
        for g in range(PER_CORE // 4):
            cn = ct = 0
            for sl in range(4):
                t = bool(tr[s + 4 * g + sl])
                ct |= (1 if t else 0) << (8 * sl)
                cn |= (0 if t else 1) << (8 * sl)
            par[0, 2 * PER_CORE + g] = cn
            par[0, 2 * PER_CORE + PER_CORE // 4 + g] = ct
        in_maps.append({
            "images": images[s:s + PER_CORE],
            "identity_in": ident,
            "gidx": _pack_gather_idx(a[s:s + PER_CORE]),
            "params": par,
        })
    return in_maps


def kernel(images, xflip_w, xflip_gate, yflip_w, yflip_gate, rot_w, rot_gate,
           trans_w, trans_gate):
    from concourse.bass_utils import run_bass_kernel_spmd

    images = np.ascontiguousarray(np.asarray(images, dtype=np.float32))
    a, b, tr = _derive_maps(xflip_w, xflip_gate, yflip_w, yflip_gate,
                            rot_w, rot_gate, trans_w, trans_gate)
    nc = _build_module()
    in_maps = _make_in_maps(images, a, b, tr)
    res = run_bass_kernel_spmd(nc, in_maps, list(range(N_CORES))).results
    return np.concatenate([res[c]["out"] for c in range(N_CORES)], axis=0)


# revision 14
# speedup vs baseline: 1.0721x; 1.0721x over previous
"""Trainium2 Bass kernel for nn_AugmentPipe (gated flips / 90-degree rots /
reflect-pad integer translation), data-parallel over the batch on 8 cores.

The whole pipeline is a per-sample separable gather:
    out[y, x, c] = in[a[y], b[x], c]            (no transpose), or
    out[y, x, c] = in[a[x], b[y], c]            (rot 90/270)
where a, b are per-sample index vectors and the transpose flag comes from
rot_w. All per-sample control (flips, rotation, translation) is folded into
(a, b, transpose) on the host; the device program is identical for every
sample so one SPMD NEFF serves all 8 cores:

  1. dma_gather: rows in[a[k], :] -> SBUF, two images per gather (halves
     gpsimd descriptor-generation instructions)
  2. column gather by b on DVE as 2 fixed copies (main + edge) whose
     source/dest element offsets are per-image registers unpacked from 2
     packed parameter words (vs 5 words: reg_load costs ~360ns/reg); b is
     always one +-1 main run (>=224) plus at most one +-1 edge run (<=32)
     from reflection padding, so padded fixed-length copies + overwrite
     order realize any b
  3. PE fp32 transpose (exact pass-through) of the gathered tile, always
  4. two cond-predicated DMA stores on SEPARATE HWDGE rings: the
     untransposed store on sync (waits only on DVE), the transposed store
     on scalar (which also does the PSUM->SBUF copies, so it follows them
     with no cross-engine wait). Keeps either store from FIFO-blocking the
     other ring behind the full per-image compute chain.
"""
import sys

for _p in ("/opt/trn_rl_repo",):
    if _p not in sys.path:
        sys.path.insert(0, _p)

import numpy as np

N_CORES = 8
N, H, W, C = 128, 256, 256, 3
PER_CORE = N // N_CORES
PAIRS = PER_CORE // 2
ROW_ELEMS = W * C  # 768
PAD = 96  # 32 pixels of slack around each data block (elements)

# M1 (gather target) free-dim layout, in elements:
#   [96 lead pad][768 h0][768 h1][96 tail pad]  -> width 1728
M1_LEAD = PAD
M1_HSTRIDE = ROW_ELEMS
M1_W = PAD + 2 * ROW_ELEMS + PAD  # 1728

# N (column-gathered) free-dim layout: [96 lead][768 h0][96 shared pad]
# [768 h1][96 tail]. Edge copies for images with no edge run dump into the
# lead pad (offset 0 writes [0,96) and [864,960) - both pads).
N_LEAD = PAD
N_HSTRIDE = ROW_ELEMS + PAD  # 864
N_W = PAD + 2 * ROW_ELEMS + 3 * PAD  # 1824

EDGE_PIX = 32
# params layout (int32 words):
#   [2*PER_CORE packed dve words][PER_CORE/4 cn-packed][PER_CORE/4 ct-packed]
NPAR_WORDS = 2 * PER_CORE + 2 * (PER_CORE // 4)


def _derive_maps(xflip_w, xflip_gate, yflip_w, yflip_gate, rot_w, rot_gate,
                 trans_w, trans_gate):
    """Replicate the reference gate logic; return (a[N,256], b[N,256], tr[N])."""
    f32 = np.float32
    n = xflip_w.shape[0]
    wx = np.where(np.asarray(xflip_gate).reshape(n) < f32(1.0),
                  np.asarray(xflip_w).reshape(n), 0)
    wy = np.where(np.asarray(yflip_gate).reshape(n) < f32(1.0),
                  np.asarray(yflip_w).reshape(n), 0)
    rw = np.where(np.asarray(rot_gate).reshape(n) < f32(1.0),
                  np.asarray(rot_w).reshape(n), 0)
    tw = np.asarray(trans_w, dtype=np.float32).reshape(2, n) * f32(2.0) - f32(1.0)
    tg = np.asarray(trans_gate).reshape(n)
    tw = np.where(tg[None, :] < f32(1.0), tw, f32(0.0)).astype(np.float32)
    tx = np.round((tw[0] * f32(W)) * f32(0.125)).astype(np.int32)
    ty = np.round((tw[1] * f32(H)) * f32(0.125)).astype(np.int32)

    idx = np.arange(W)
    xi = (W - 1) - np.abs((W - 1) - (idx[None, :] - tx[:, None]) % (2 * W - 2))
    yi = (H - 1) - np.abs((H - 1) - (idx[None, :] + ty[:, None]) % (2 * H - 2))

    xftot = (wx == 1) ^ ((rw == 1) | (rw == 2))
    yftot = (wy == 1) ^ ((rw == 2) | (rw == 3))
    tr = (rw == 1) | (rw == 3)

    a = np.where(tr[:, None], xi, yi)
    a = np.where(yftot[:, None], (H - 1) - a, a)
    b = np.where(tr[:, None], yi, xi)
    b = np.where(xftot[:, None], (W - 1) - b, b)
    return a.astype(np.int64), b.astype(np.int64), tr


def _fit_template(b):
    """Fit b (one +-1 main run >=224 plus <=1 edge run <=32) to the fixed
    2-copy template. Returns 2 packed int32 words:
      word0 = m_src | m_dst << 16
      word1 = e_src | e_dst << 16 | R << 31     (R=1 -> descending main)

    Both reversing copies are written forward-read / backward-write (the
    backward-READ [-3,n],[1,3] pattern runs at ~0.29 elem/cycle on DVE,
    3.6x slower than contiguous; forward reads stream at ~1/cycle):
      R=0: asc main src[1,768] -> dst[1,768]; desc edge src[1,96] ->
           dst[-3,32],[1,3] anchored at the window's LAST pixel
      R=1: desc main src[1,768] -> dst[-3,256],[1,3] anchored at pixel
           mp+255 (excess lands in pads / edge-overwritten region);
           asc edge src[1,96] -> dst[1,96]
    """
    base = M1_LEAD
    d = np.diff(b)
    assert np.all(np.abs(d) == 1), b
    change = np.nonzero(d[1:] != d[:-1])[0]
    assert len(change) <= 1, b
    if len(change) == 0:
        runs = [(0, W, int(d[0]))]
    else:
        # the pivot position can belong to either run; pick the split whose
        # short run is <= EDGE_PIX
        c0 = int(change[0])
        runs = None
        for cut in (c0 + 1, c0 + 2):
            r = [(0, cut, int(d[0])), (cut, W, int(d[cut]))]
            lens = sorted(e - s for s, e, _ in r)
            if lens[0] <= EDGE_PIX and lens[1] >= W - EDGE_PIX:
                runs = r
                break
        assert runs is not None, (b, c0)
    if len(runs) == 1:
        main, edge = runs[0], None
    else:
        r0, r1 = runs
        if (r0[1] - r0[0]) >= (r1[1] - r1[0]):
            main, edge = r0, r1
        else:
            main, edge = r1, r0
    mp, mq, md = main
    assert mq - mp >= W - EDGE_PIX, (b, runs)

    # main direction decides the branch: R=0 -> asc main + desc edge,
    # R=1 -> desc main + asc edge
    R = 0 if md == 1 else 1
    if R == 0:
        m_src = base + 3 * int(b[mp])
        m_dst = N_LEAD + 3 * mp
    else:
        # backward-write anchor: dst pixel mp+255 descending, src forward
        m_src = base + 3 * (int(b[mp]) - (W - 1))
        m_dst = N_LEAD + 3 * (mp + (W - 1))

    if edge is not None:
        ep, eq, ed = edge
        assert eq - ep <= EDGE_PIX and ed == -md, (b, runs)
        if ep == 0:
            wstart = eq - EDGE_PIX  # head edge: window [eq-32, eq)
        else:
            assert eq == W, (b, runs)
            wstart = ep             # tail edge: window [ep, ep+32)
        v0 = int(b[ep]) + ed * (wstart - ep)  # value at window start
        if R == 0:
            # desc edge, backward-write: anchor at window's last pixel
            e_src = base + 3 * (v0 - (EDGE_PIX - 1))
            e_dst = N_LEAD + 3 * (wstart + (EDGE_PIX - 1))
        else:
            # asc edge, forward/forward
            e_src = base + 3 * v0
            e_dst = N_LEAD + 3 * wstart
        assert e_src >= 0 and e_dst > 0, (b, runs, e_src, e_dst)
    else:
        # taken branch's edge copy still runs; dump into the N pads
        # (R=0 writes backward from 93, R=1 forward from 0 - both land in
        # the lead pad [0,96) and the shared pad [864,960))
        e_src = base
        e_dst = 93 if R == 0 else 0

    for v in (m_src, m_dst, e_src, e_dst):
        assert 0 <= v < (1 << 15), (m_src, m_dst, e_src, e_dst)
    w0 = m_src | (m_dst << 16)
    w1 = e_src | (e_dst << 16) | (R << 31)
    return [w0, w1]


def _pack_gather_idx(a_core):
    """a_core: [PER_CORE, 256] row indices -> int16 [128, 16*PER_CORE] in
    dma_gather layout (index i at partition i%16, col i//16, replicated to
    all 8 gpsimd core partition groups)."""
    out = np.zeros((128, 16 * PER_CORE), np.int16)
    for img in range(PER_CORE):
        blk = a_core[img].astype(np.int16).reshape(16, 16).T  # [p=i%16, s=i//16]
        for g in range(8):
            out[16 * g:16 * (g + 1), 16 * img:16 * (img + 1)] = blk
    return out


_NC_CACHE = {}


def _build_module(coresim_pads=False):
    key = ("nc", coresim_pads)
    if key in _NC_CACHE:
        return _NC_CACHE[key]
    import concourse.bacc as bacc
    import concourse.bass as bass
    import concourse.mybir as mybir
    import concourse.tile as tile
    from concourse.ap import AP

    DT = mybir.dt.float32
    I32 = mybir.dt.int32
    ALU = mybir.AluOpType
    nc = bacc.Bacc(None, num_swdge_queues=2)
    images = nc.dram_tensor("images", [PER_CORE, H, W, C], DT, kind="ExternalInput")
    identity_in = nc.dram_tensor("identity_in", [128, 128], DT, kind="ExternalInput")
    gidx = nc.dram_tensor("gidx", [128, 16 * PER_CORE], mybir.dt.int16,
                          kind="ExternalInput")
    params = nc.dram_tensor("params", [1, NPAR_WORDS], I32, kind="ExternalInput")
    out = nc.dram_tensor("out", [PER_CORE, H, W, C], DT, kind="ExternalOutput")

    img_elems = H * W * C

    with tile.TileContext(nc) as tc:
        with (
            tc.tile_pool(name="const", bufs=1) as const_pool,
            tc.tile_pool(name="m1", bufs=8) as m1_pool,
            tc.tile_pool(name="ncg", bufs=8) as n_pool,
            tc.tile_pool(name="tt", bufs=6) as t_pool,
            tc.tile_pool(name="psum", bufs=4, space="PSUM") as psum_pool,
        ):
            # warmup: a throwaway 16-row gather issued first so the gpsimd
            # ucode library load (ModifyPoolConfig + ~8us DMA) overlaps the
            # preamble instead of gating the first real gather
            warm_idx = const_pool.tile([128, 1], mybir.dt.int16)
            nc.gpsimd.memset(warm_idx[:], 0)
            warm_out = const_pool.tile([128, 1, ROW_ELEMS], DT)
            nc.gpsimd.dma_gather(
                warm_out[:], AP(images[:].tensor, 0, [[ROW_ELEMS, H], [1, ROW_ELEMS]]),
                warm_idx[:], num_idxs=16, num_idxs_reg=16, elem_size=ROW_ELEMS,
                queue_num=0, single_packet=False)

            idx_t = const_pool.tile([128, 16 * PER_CORE], mybir.dt.int16)
            nc.sync.dma_start(idx_t[:], gidx[:])
            par_t = const_pool.tile([1, NPAR_WORDS], I32)
            nc.sync.dma_start(par_t[:], params[:])
            ident = const_pool.tile([128, 128], DT)
            nc.sync.dma_start(ident[:], identity_in[:])

            dve = nc.vector.engine
            act = nc.scalar.engine
            sp = nc.sync.engine

            # per-4-image packed store-cond registers, one stream per engine
            cn_group = [None]
            ct_group = [None]

            m1_tiles = [None] * PER_CORE

            def emit_gather(k):
                m1 = m1_pool.tile([128, M1_W], DT, tag="m1")
                m1_tiles[k] = m1
                if coresim_pads:
                    nc.gpsimd.memset(m1[:, 0:M1_LEAD], 0.0)
                    nc.gpsimd.memset(m1[:, M1_W - PAD:M1_W], 0.0)
                src = AP(images[:].tensor, k * img_elems,
                         [[ROW_ELEMS, H], [1, ROW_ELEMS]])
                gout = m1[:, M1_LEAD:M1_LEAD + 2 * ROW_ELEMS].rearrange(
                    "p (h e) -> p h e", h=2)
                nc.gpsimd.dma_gather(
                    gout, src, idx_t[:, 16 * k:16 * (k + 1)],
                    num_idxs=H, num_idxs_reg=H, elem_size=ROW_ELEMS,
                    queue_num=k % 2, single_packet=False)

            for k in range(4):
                emit_gather(k)

            for i in range(PER_CORE):
                if i + 4 < PER_CORE:
                    emit_gather(i + 4)
                m1 = m1_tiles[i]

                # --- 2. column gather by b: M1 -> Ntile (2 reg-offset
                # copies selected by the R sign-bit branch) ---
                ntile = n_pool.tile([128, N_W], DT, tag="ncg")
                m1t, ntt = m1[:].tensor, ntile[:].tensor
                p_m1 = [M1_W, 128]
                p_n = [N_W, 128]
                w0 = nc.alloc_register(dve, f"cg{i}_w0")
                w1 = nc.alloc_register(dve, f"cg{i}_w1")
                nc.vector.reg_load([w0, w1], par_t[0:1, 2 * i:2 * i + 2])
                m_src = nc.alloc_register(dve, f"cg{i}_ms")
                m_dst = nc.alloc_register(dve, f"cg{i}_md")
                e_src = nc.alloc_register(dve, f"cg{i}_es")
                e_dst = nc.alloc_register(dve, f"cg{i}_ed")
                nc.vector.reg_alu(m_src, bass.RuntimeValue(w0), 0xFFFF,
                                  ALU.bitwise_and)
                nc.vector.reg_alu(m_dst, bass.RuntimeValue(w0), 16,
                                  ALU.logical_shift_right)
                nc.vector.reg_alu(e_src, bass.RuntimeValue(w1), 0xFFFF,
                                  ALU.bitwise_and)
                nc.vector.reg_alu(e_dst, bass.RuntimeValue(w1), 16,
                                  ALU.logical_shift_right)
                nc.vector.reg_alu(e_dst, bass.RuntimeValue(e_dst), 0x7FFF,
                                  ALU.bitwise_and)
                with tc.If(bass.RuntimeValue(w1) >= 0) as cmp:
                    # R=0: ascending main (fwd/fwd) + descending edge
                    # (fwd read, backward write)
                    nc.vector.tensor_copy(
                        AP(ntt, m_dst, [p_n, [N_HSTRIDE, 2], [1, ROW_ELEMS]]),
                        AP(m1t, m_src, [p_m1, [M1_HSTRIDE, 2], [1, ROW_ELEMS]]))
                    nc.vector.tensor_copy(
                        AP(ntt, e_dst, [p_n, [N_HSTRIDE, 2], [-3, EDGE_PIX], [1, C]]),
                        AP(m1t, e_src, [p_m1, [M1_HSTRIDE, 2], [1, 3 * EDGE_PIX]]))
                with cmp.Else():
                    # R=1: descending main (fwd read, backward write) +
                    # ascending edge (fwd/fwd)
                    nc.vector.tensor_copy(
                        AP(ntt, m_dst, [p_n, [N_HSTRIDE, 2], [-3, W], [1, C]]),
                        AP(m1t, m_src, [p_m1, [M1_HSTRIDE, 2], [1, ROW_ELEMS]]))
                    nc.vector.tensor_copy(
                        AP(ntt, e_dst, [p_n, [N_HSTRIDE, 2], [1, 3 * EDGE_PIX]]),
                        AP(m1t, e_src, [p_m1, [M1_HSTRIDE, 2], [1, 3 * EDGE_PIX]]))

                # --- 3. pixel transpose Ntile -> Ttile via PE (exact fp32) ---
                # 6 channel transposes interleave into one 2-bank PSUM tile
                # (hu blocks at 512-elem, i.e. bank-aligned, offsets); a single
                # strided copy on the scalar engine moves each [128, 2, 384]
                # double-block out
                ttile = t_pool.tile([128, 2, ROW_ELEMS], DT, tag="tt")
                ttt = ttile[:].tensor
                for hk in range(2):
                    pt = psum_pool.tile([128, 1024], DT, tag="pt")
                    ptt = pt[:].tensor
                    for hu in range(2):
                        for c in range(C):
                            stat = AP(ntt, N_LEAD + hk * N_HSTRIDE + 3 * (hu * 128) + c,
                                      [p_n, [3, 128]])
                            nc.tensor.transpose(
                                AP(ptt, 512 * hu + c, [[1024, 128], [3, 128]]),
                                stat, ident[:])
                    t0 = 3 * (hk * 128)
                    nc.scalar.copy(
                        AP(ttt, t0, [[2 * ROW_ELEMS, 128], [ROW_ELEMS, 2],
                                     [1, 3 * 128]]),
                        AP(ptt, 0, [[1024, 128], [512, 2], [1, 3 * 128]]))

                # --- 4. predicated stores, one per HWDGE ring ---
                dram_out = AP(out[:].tensor, i * img_elems,
                              [[ROW_ELEMS, 128], [128 * ROW_ELEMS, 2], [1, ROW_ELEMS]])
                n_src = AP(ntt, N_LEAD, [p_n, [N_HSTRIDE, 2], [1, ROW_ELEMS]])
                g, sl = i // 4, i % 4
                if sl == 0:
                    cn_group[0] = nc.alloc_register(sp, f"cn_g{g}")
                    nc.sync.reg_load([cn_group[0]],
                                     par_t[0:1, 2 * PER_CORE + g:2 * PER_CORE + g + 1])
                    ct_group[0] = nc.alloc_register(act, f"ct_g{g}")
                    nc.scalar.reg_load(
                        [ct_group[0]],
                        par_t[0:1, 2 * PER_CORE + PER_CORE // 4 + g:
                              2 * PER_CORE + PER_CORE // 4 + g + 1])
                cn_reg = nc.alloc_register(sp, f"cn_{i}")
                nc.sync.reg_alu(cn_reg, bass.RuntimeValue(cn_group[0]), 8 * sl,
                                ALU.logical_shift_right)
                nc.sync.reg_alu(cn_reg, bass.RuntimeValue(cn_reg), 1,
                                ALU.bitwise_and)
                cn = nc.sync.snap(cn_reg, min_val=0, max_val=1)
                nc.sync.dma_start(dram_out, n_src, cond=cn)

                ct_reg = nc.alloc_register(act, f"ct_{i}")
                nc.scalar.reg_alu(ct_reg, bass.RuntimeValue(ct_group[0]), 8 * sl,
                                  ALU.logical_shift_right)
                nc.scalar.reg_alu(ct_reg, bass.RuntimeValue(ct_reg), 1,
                                  ALU.bitwise_and)
                ct = nc.scalar.snap(ct_reg, min_val=0, max_val=1)
                nc.scalar.dma_start(dram_out.copy(), ttile[:], cond=ct)

    nc.finalize()
    _NC_CACHE[key] = nc
    return nc


def _make_in_maps(images, a, b, tr):
    ident = np.eye(128, dtype=np.float32)
    in_maps = []
    for core in range(N_CORES):
        s = core * PER_CORE
        par = np.zeros((1, NPAR_WORDS), np.int32)
        for i in range(PER_CORE):
            par[0, 2 * i:2 * i + 2] = np.array(
                _fit_template(b[s + i]), np.uint32).astype(np.int32)
        for g in range(PER_CORE // 4):
            cn = ct = 0
            for sl in range(4):
                t = bool(tr[s + 4 * g + sl])
                ct |= (1 if t else 0) << (8 * sl)
                cn |= (0 if t else 1) << (8 * sl)
            par[0, 2 * PER_CORE + g] = cn
            par[0, 2 * PER_CORE + PER_CORE // 4 + g] = ct
        in_maps.append({
            "images": images[s:s + PER_CORE],
            "identity_in": ident,
            "gidx": _pack_gather_idx(a[s:s + PER_CORE]),
            "params": par,
        })
    return in_maps


def kernel(images, xflip_w, xflip_gate, yflip_w, yflip_gate, rot_w, rot_gate,
           trans_w, trans_gate):
    from concourse.bass_utils import run_bass_kernel_spmd

    images = np.ascontiguousarray(np.asarray(images, dtype=np.float32))
    a, b, tr = _derive_maps(xflip_w, xflip_gate, yflip_w, yflip_gate,
                            rot_w, rot_gate, trans_w, trans_gate)
    nc = _build_module()
    in_maps = _make_in_maps(images, a, b, tr)
    res = run_bass_kernel_spmd(nc, in_maps, list(range(N_CORES))).results
    return np.concatenate([res[c]["out"] for c in range(N_CORES)], axis=0)
